# revision 12
# baseline (speedup 1.0000x reference)
# Trainium2 Bass kernel for nn_MultiHeadAttention_85933705658435
#
# Reference: LayerNorm(scale-only) -> QKV projection -> full softmax attention
#            -> output projection.  x:[S,B,E] f32, S=2048 B=2 E=1024, H=16 D=64.
#
# Sharding (8 cores): batch (2-way) x head-group (4-way, 4 heads/core).
#   - W_qkv column-sharded (the core's 4 heads), W_out row-sharded; the 4
#     partial output projections per batch are summed on the host.
#
# v2 structure (vs the 319us baseline):
#   - LN -> DMA-transpose -> QKV runs as a per-512-token-superchunk pipeline,
#     so the PE starts QKV ~12us in instead of idling 67us behind a fully
#     serialized LN phase.
#   - V is computed token-major directly (lnT chunk as the stationary
#     operand), killing the V DRAM bounce + 16 transposes of the baseline.
#   - Attention is emitted per 512-query tile with the two head-pairs'
#     scores+exp interleaved, so ACT (the exp engine, the true bottleneck at
#     ~1.15us per [128,1024] exp) starts as soon as superchunk 0's K/Q exist
#     and never waits for the full QKV phase.
#   - exp writes fp8e4 pairs; ctx uses fp8 DoubleRow matmuls (2 key-chunks
#     contracted per instruction) with the appended-ones column still giving
#     the softmax denominators for free.
#   - out-projection of q-tile qt overlaps the attention of qt+1 (its own
#     2 PSUM banks; scores 4 + ctx 2 + outproj 2 = 8 banks exactly).

import numpy as np
import ml_dtypes

S, B, E = 2048, 2, 1024
H, D = 16, 64
HPC = 4              # heads per core
NCORES = 8
EPS = 1e-6
FQK = HPC * D        # 256 (per-core Q width = K width = V width)
P = 128
TC = S // P          # 16 token chunks
ECH = E // P         # 8 e-chunks
NSC = 4              # superchunks in the front pipeline
SCT = S // NSC       # 512 tokens per superchunk
QTS = 512            # q-tile size in attention
NQT = S // QTS       # 4
NKP = TC // 2        # 8 key-chunk pairs

CTX_FP8 = False      # fp8e4 exp + V'
CTX_DR = False       # DoubleRow ctx matmuls (requires CTX_FP8)
EXP_SHIFT = -1.6 if CTX_FP8 else 0.0

BF16 = ml_dtypes.bfloat16

_CACHE = {}


def _build_nc():
    from contextlib import ExitStack

    import concourse.bass as bass
    import concourse.tile as tile
    from concourse import bacc, mybir
    from concourse.tile import add_dep_helper

    dt = mybir.dt
    Alu = mybir.AluOpType
    Act = mybir.ActivationFunctionType
    EDT = dt.float8e4 if CTX_FP8 else dt.bfloat16
    DR = mybir.MatmulPerfMode.DoubleRow

    nc = bacc.Bacc(trn_type="TRN2")
    x_d = nc.dram_tensor("x", (S, E), dt.float32, kind="ExternalInput").ap()
    # wqkv: [E, 3*FQK] = Q | K | V column blocks for this core's 4 heads
    wqkv_d = nc.dram_tensor(
        "wqkv", (E, 3 * FQK), dt.bfloat16, kind="ExternalInput"
    ).ap()
    wo_d = nc.dram_tensor("wo", (FQK, E), dt.bfloat16, kind="ExternalInput").ap()
    out_d = nc.dram_tensor("out", (S, E), dt.float32, kind="ExternalOutput").ap()

    with tile.TileContext(nc) as tc, ExitStack() as ctx:
        singles = ctx.enter_context(tc.tile_pool(name="singles", bufs=1))
        xp = ctx.enter_context(tc.tile_pool(name="xp", bufs=3))
        lnp = ctx.enter_context(tc.tile_pool(name="lnp", bufs=3))
        small = ctx.enter_context(tc.tile_pool(name="small", bufs=4))
        evac = ctx.enter_context(tc.tile_pool(name="evac", bufs=2))
        dram = ctx.enter_context(tc.tile_pool(name="dram", bufs=1, space="DRAM"))

        # persistent SBUF tensors
        lnT = singles.tile([P, ECH, S], dt.bfloat16)          # ln^T, e-chunked
        qkT = singles.tile([P, 4, S], dt.bfloat16)            # fc 0,1: Q^T; 2,3: K^T
        # token-major V (+ ones col at 64) per t-chunk / head; 68 pad so the
        # DoubleRow k-pair stride (4*68) is a multiple of 16 bytes
        Vp = singles.tile([P, TC, HPC, 68], EDT)
        # exp staging: (pr, kcp, head, parity, q); parity = kc&1 so a kcp
        # slice is the [Ki, Ko=2, N] moving operand of the DoubleRow ctx mm
        eP = singles.tile([P, 2, NKP, 2, 2, QTS], EDT)
        ones_dr = singles.tile([P, 2, 1], EDT)                # denominator lhsT
        w_sb = singles.tile([P, ECH, 3 * FQK], dt.bfloat16)
        wo_sb = singles.tile([P, 2, E], dt.bfloat16)
        eps_sb = singles.tile([P, 1], dt.float32)
        ctxn = singles.tile([P, 2, S], dt.bfloat16)           # normalized ctx^T
        ln_dram = dram.tile([S, E], dt.bfloat16)
        rc_dram = dram.tile([NQT * 4, QTS], dt.float32)

        shf_sb = singles.tile([P, 1], dt.float32)
        nc.vector.memset(shf_sb[:], EXP_SHIFT)
        nc.vector.memset(eps_sb[:], EPS)
        nc.sync.dma_start(w_sb[:], wqkv_d.rearrange("(c p) f -> p c f", p=P))
        nc.sync.dma_start(wo_sb[:], wo_d.rearrange("(c p) e -> p c e", p=P))
        nc.vector.memset(Vp[:], 0.0)
        nc.vector.memset(Vp[:, :, :, 64:65], 1.0)
        nc.vector.memset(ones_dr[:], 1.0)

        # ---- front: LN -> transpose -> QKV, pipelined per superchunk ------
        with tc.tile_pool(name="psF", bufs=2, space="PSUM") as psF:
            # HAM warmup while LN(sc0) runs on DVE/DMA
            wps = psF.tile([P, 512], dt.float32, tag="qkv", name="wps")
            for _ in range(20):
                nc.tensor.matmul(
                    wps[:], w_sb[:, 0, 0:P], w_sb[:, 0, 0:512],
                    start=True, stop=True,
                )

            for sc in range(NSC):
                t0, t1 = sc * (TC // NSC), (sc + 1) * (TC // NSC)
                ln_writes = []
                for t in range(t0, t1):
                    xb = xp.tile([P, E], dt.float32, tag="xb")
                    # x loads + ln bounce on the ACT hwdge queue (plain
                    # copies are safe there; transposes are not)
                    nc.scalar.dma_start(xb[:], x_d[t * P : (t + 1) * P, :])
                    st = small.tile([P, 2, 6], dt.float32, tag="st")
                    nc.vector.bn_stats(st[:, 0, :], xb[:, 0:512])
                    nc.vector.bn_stats(st[:, 1, :], xb[:, 512:1024])
                    mv = small.tile([P, 2], dt.float32, tag="mv")
                    nc.vector.bn_aggr(mv[:], st[:])
                    sd = small.tile([P, 1], dt.float32, tag="sd")
                    nc.scalar.activation(sd[:], mv[:, 1:2], Act.Sqrt, bias=eps_sb[:])
                    rs = small.tile([P, 1], dt.float32, tag="rs")
                    nc.vector.reciprocal(rs[:], sd[:])
                    lnb = lnp.tile([P, E], dt.bfloat16, tag="lnb")
                    nc.vector.tensor_scalar(
                        lnb[:], xb[:], mv[:, 0:1], rs[:], Alu.subtract, Alu.mult
                    )
                    # ln bounce on the SWDGE (gpsimd) queue: the scalar queue
                    # must stay free-running for x loads (a write waiting on
                    # the LN compute chain would stall later x loads behind
                    # it), and the sync queue is dispatch-bound on transposes
                    ln_writes.append(
                        nc.gpsimd.dma_start(ln_dram[t * P : (t + 1) * P, :], lnb[:])
                    )
                # transpose this superchunk (DRAM pool tiles are not
                # dependency-tracked; add the RAW edges explicitly)
                for c in range(ECH):
                    tp = nc.sync.dma_start_transpose(
                        lnT[:, c, sc * SCT : (sc + 1) * SCT],
                        ln_dram[sc * SCT : (sc + 1) * SCT, c * P : (c + 1) * P],
                    )
                    for wi in ln_writes:
                        add_dep_helper(tp.ins, wi.ins, True, "lnT RAW via ln_dram")
                # K then Q, feature-major, weights-stationary
                for fc in (2, 3, 0, 1):
                    ps = psF.tile([P, 512], dt.float32, tag="qkv")
                    for ec in range(ECH):
                        nc.tensor.matmul(
                            ps[:],
                            w_sb[:, ec, fc * P : (fc + 1) * P],
                            lnT[:, ec, sc * SCT : (sc + 1) * SCT],
                            start=(ec == 0),
                            stop=(ec == ECH - 1),
                        )
                    nc.vector.tensor_copy(qkT[:, fc, sc * SCT : (sc + 1) * SCT], ps[:])
                # V token-major: lnT chunk stationary, V weight cols moving
                for t in range(t0, t1):
                    vps = psF.tile([P, FQK], dt.float32, tag="vtm")
                    for ec in range(ECH):
                        nc.tensor.matmul(
                            vps[:],
                            lnT[:, ec, t * P : (t + 1) * P],
                            w_sb[:, ec, 2 * FQK : 3 * FQK],
                            start=(ec == 0),
                            stop=(ec == ECH - 1),
                        )
                    nc.vector.tensor_copy(
                        Vp[:, t, :, 0:64],
                        vps[:].rearrange("p (h d) -> p h d", d=64),
                    )

        # ---- attention: per q-tile, both head-pairs' exp interleaved ------
        ctx2 = ExitStack()
        with ctx2:
            psS = ctx2.enter_context(tc.tile_pool(name="psS", bufs=2, space="PSUM"))
            psC = ctx2.enter_context(tc.tile_pool(name="psC", bufs=1, space="PSUM"))
            psO = ctx2.enter_context(tc.tile_pool(name="psO", bufs=1, space="PSUM"))

            def scores_exp(pr, qt, kc):
                q0 = qt * QTS
                k0 = kc * P
                kcp, par = divmod(kc, 2)
                sq = psS.tile([P, 2 * QTS], dt.float32, tag="sq", name="sq")
                nc.tensor.matmul(
                    sq[:, 0:QTS],
                    qkT[0:64, 2 + pr, k0 : k0 + P],
                    qkT[0:64, pr, q0 : q0 + QTS],
                    start=True, stop=True, tile_position=(0, 0),
                )
                nc.tensor.matmul(
                    sq[:, QTS : 2 * QTS],
                    qkT[64:128, 2 + pr, k0 : k0 + P],
                    qkT[64:128, pr, q0 : q0 + QTS],
                    start=True, stop=True, tile_position=(64, 0),
                )
                # exp(s - C): softmax is shift-invariant (the ones-column
                # denominator uses the same shifted values).  The shift keeps
                # the heavy score tail under fp8e4m3's 448 max; the resulting
                # underflow of weights below ~e^-6 costs <0.2% of the
                # denominator mass.
                nc.scalar.activation(
                    eP[:, pr, kcp, :, par, :],
                    sq[:].rearrange("p (h q) -> p h q", h=2),
                    Act.Exp,
                    bias=shf_sb[:],
                )

            def ctx_block(pr, hh_base):
                # 8 DoubleRow matmuls per head: 2 key-chunks per instruction;
                # row 64 of the output accumulates the softmax denominator
                cps = []
                for h in range(2):
                    cp = psC.tile([65, QTS], dt.float32, tag=f"cps{h}",
                                  name=f"cps{h}")
                    cps.append(cp)
                    for kcp in range(NKP):
                        if CTX_DR:
                            nc.tensor.matmul(
                                cp[:],
                                Vp[:, 2 * kcp : 2 * kcp + 2, hh_base + h, 0:65],
                                eP[:, pr, kcp, h, :, :],
                                start=(kcp == 0), stop=(kcp == NKP - 1),
                                perf_mode=DR,
                            )
                        else:
                            for par in range(2):
                                nc.tensor.matmul(
                                    cp[:],
                                    Vp[:, 2 * kcp + par, hh_base + h, 0:65],
                                    eP[:, pr, kcp, h, par, :],
                                    start=(kcp == 0 and par == 0),
                                    stop=(kcp == NKP - 1 and par == 1),
                                )
                return cps

            def normalize(pr, qt, cps):
                # ctx^T[d,q] / denom[q]; denom is ctx row 64.  reciprocal of
                # a [1,512] row is ~3us on one DVE lane, so reshape to
                # [128,4] via a small DMA, recip, then broadcast across 64
                # partitions with a step-0 DMA read from DRAM.
                q0 = qt * QTS
                ctxu = []
                for h in range(2):
                    cu = evac.tile([65, QTS], dt.float32, tag=f"ctxu{h}",
                                   name=f"ctxu{h}")
                    nc.vector.tensor_copy(cu[:], cps[h][:])
                    ctxu.append(cu)
                for h in range(2):
                    dnp = small.tile([P, QTS // P], dt.float32, tag="dnp")
                    nc.sync.dma_start(dnp[:], ctxu[h][64:65, :])
                    rcp = small.tile([P, QTS // P], dt.float32, tag="rcp")
                    nc.vector.reciprocal(rcp[:], dnp[:])
                    slot = (qt * 2 + pr) * 2 + h
                    rc_row = rc_dram[slot : slot + 1, :]
                    wr = nc.sync.dma_start(rc_row, rcp[:])
                    bcs = evac.tile([64, QTS], dt.float32, tag="bcs")
                    rc_bcast = bass.AP(
                        tensor=rc_row.tensor,
                        offset=rc_row.offset,
                        ap=[[0, 64]] + list(rc_row.ap[1:]),
                    )
                    rd = nc.sync.dma_start(bcs[:], rc_bcast)
                    add_dep_helper(rd.ins, wr.ins, True, "recip RAW via dram")
                    if h == 0:
                        nc.vector.tensor_tensor(
                            ctxn[0:64, pr, q0 : q0 + QTS],
                            ctxu[0][0:64, :], bcs[:], Alu.mult,
                        )
                    else:
                        tmpn = evac.tile([64, QTS], dt.bfloat16, tag="tmpn")
                        nc.vector.tensor_tensor(
                            tmpn[:], ctxu[1][0:64, :], bcs[:], Alu.mult
                        )
                        # partition shift 0-63 -> 64-127 via SBUF-SBUF DMA
                        nc.sync.dma_start(ctxn[64:128, pr, q0 : q0 + QTS], tmpn[:])

            def outproj(qt):
                for t in range(qt * (QTS // P), (qt + 1) * (QTS // P)):
                    po = psO.tile([P, E], dt.float32, tag="po")
                    for et in range(2):
                        for pr in range(2):
                            nc.tensor.matmul(
                                po[:, et * 512 : (et + 1) * 512],
                                ctxn[:, pr, t * P : (t + 1) * P],
                                wo_sb[:, pr, et * 512 : (et + 1) * 512],
                                start=(pr == 0), stop=(pr == 1),
                            )
                    ob = evac.tile([P, E], dt.float32, tag="ob", bufs=3)
                    nc.vector.tensor_copy(ob[:], po[:])
                    nc.scalar.dma_start(out_d[t * P : (t + 1) * P, :], ob[:])

            # outproj(qt) is EMITTED a few kc into round qt+1: the PE queue
            # is a static FIFO, and outproj's ctxn dependency sits behind a
            # multi-DMA normalize chain -- placing it before the next
            # round's scores would stall the scores and starve ACT
            pend = None
            for qt in range(NQT):
                for kc in range(TC):
                    scores_exp(0, qt, kc)
                    scores_exp(1, qt, kc)
                    if kc == 5 and pend is not None:
                        outproj(pend)
                        pend = None
                for pr in range(2):
                    cps = ctx_block(pr, pr * 2)
                    normalize(pr, qt, cps)
                pend = qt
            outproj(pend)

    nc.compile()
    return nc


def make_in_maps(x, ln_scale, w_qkv, w_out):
    w = (np.asarray(w_qkv, np.float32) * np.asarray(ln_scale, np.float32)[:, None])
    wo = np.asarray(w_out, np.float32)
    in_maps = []
    for c in range(NCORES):
        b, g = divmod(c, 4)
        h0 = g * HPC
        wq = w[:, h0 * D : (h0 + HPC) * D]
        wk = w[:, H * D + h0 * D : H * D + (h0 + HPC) * D]
        wv = w[:, 2 * H * D + h0 * D : 2 * H * D + (h0 + HPC) * D]
        in_maps.append(
            {
                "x": np.ascontiguousarray(np.asarray(x, np.float32)[:, b, :]),
                "wqkv": np.ascontiguousarray(
                    np.concatenate([wq, wk, wv], axis=1)
                ).astype(BF16),
                "wo": np.ascontiguousarray(
                    wo[h0 * D : (h0 + HPC) * D, :]
                ).astype(BF16),
            }
        )
    return in_maps


def get_nc():
    if "nc" not in _CACHE:
        _CACHE["nc"] = _build_nc()
    return _CACHE["nc"]


def assemble(results):
    out = np.empty((S, B, E), np.float32)
    for b in range(B):
        acc = results[4 * b]["out"].astype(np.float32).copy()
        for g in range(1, 4):
            acc += results[4 * b + g]["out"]
        out[:, b, :] = acc
    return out


def kernel(x, ln_scale, w_qkv, w_out):
    from concourse.bass_utils import run_bass_kernel_spmd

    nc = get_nc()
    in_maps = make_in_maps(x, ln_scale, w_qkv, w_out)
    res = run_bass_kernel_spmd(nc, in_maps, core_ids=list(range(NCORES)))
    return assemble(res.results)


# revision 17
# speedup vs baseline: 1.0448x; 1.0448x over previous
# Trainium2 Bass kernel for nn_MultiHeadAttention_85933705658435
#
# Reference: LayerNorm(scale-only) -> QKV projection -> full softmax attention
#            -> output projection.  x:[S,B,E] f32, S=2048 B=2 E=1024, H=16 D=64.
#
# Sharding (8 cores): batch (2-way) x head-group (4-way, 4 heads/core).
#   - W_qkv column-sharded (the core's 4 heads), W_out row-sharded; the 4
#     partial output projections per batch are summed on the host.
#
# v2 structure (vs the 319us baseline):
#   - LN -> DMA-transpose -> QKV runs as a per-512-token-superchunk pipeline,
#     so the PE starts QKV ~12us in instead of idling 67us behind a fully
#     serialized LN phase.
#   - V is computed token-major directly (lnT chunk as the stationary
#     operand), killing the V DRAM bounce + 16 transposes of the baseline.
#   - Attention is emitted per 512-query tile with the two head-pairs'
#     scores+exp interleaved, so ACT (the exp engine, the true bottleneck at
#     ~1.15us per [128,1024] exp) starts as soon as superchunk 0's K/Q exist
#     and never waits for the full QKV phase.
#   - exp writes fp8e4 pairs; ctx uses fp8 DoubleRow matmuls (2 key-chunks
#     contracted per instruction) with the appended-ones column still giving
#     the softmax denominators for free.
#   - out-projection of q-tile qt overlaps the attention of qt+1 (its own
#     2 PSUM banks; scores 4 + ctx 2 + outproj 2 = 8 banks exactly).

import numpy as np
import ml_dtypes

S, B, E = 2048, 2, 1024
H, D = 16, 64
HPC = 4              # heads per core
NCORES = 8
EPS = 1e-6
FQK = HPC * D        # 256 (per-core Q width = K width = V width)
P = 128
TC = S // P          # 16 token chunks
ECH = E // P         # 8 e-chunks
NSC = 4              # superchunks in the front pipeline
SCT = S // NSC       # 512 tokens per superchunk
QTS = 512            # q-tile size in attention
NQT = S // QTS       # 4
NKP = TC // 2        # 8 key-chunk pairs

CTX_FP8 = False      # fp8e4 exp + V'
CTX_DR = False       # DoubleRow ctx matmuls (requires CTX_FP8)
EXP_SHIFT = -1.6 if CTX_FP8 else 0.0

BF16 = ml_dtypes.bfloat16

_CACHE = {}


def _build_nc():
    from contextlib import ExitStack

    import concourse.bass as bass
    import concourse.tile as tile
    from concourse import bacc, mybir
    from concourse.tile import add_dep_helper

    dt = mybir.dt
    Alu = mybir.AluOpType
    Act = mybir.ActivationFunctionType
    EDT = dt.float8e4 if CTX_FP8 else dt.bfloat16
    DR = mybir.MatmulPerfMode.DoubleRow

    nc = bacc.Bacc(trn_type="TRN2")
    x_d = nc.dram_tensor("x", (S, E), dt.float32, kind="ExternalInput").ap()
    # wqkv: [E, 3*FQK] = Q | K | V column blocks for this core's 4 heads
    wqkv_d = nc.dram_tensor(
        "wqkv", (E, 3 * FQK), dt.bfloat16, kind="ExternalInput"
    ).ap()
    wo_d = nc.dram_tensor("wo", (FQK, E), dt.bfloat16, kind="ExternalInput").ap()
    out_d = nc.dram_tensor("out", (S, E), dt.float32, kind="ExternalOutput").ap()

    with tile.TileContext(nc) as tc, ExitStack() as ctx:
        singles = ctx.enter_context(tc.tile_pool(name="singles", bufs=1))
        xp = ctx.enter_context(tc.tile_pool(name="xp", bufs=3))
        lnp = ctx.enter_context(tc.tile_pool(name="lnp", bufs=3))
        small = ctx.enter_context(tc.tile_pool(name="small", bufs=4))
        evac = ctx.enter_context(tc.tile_pool(name="evac", bufs=2))
        dram = ctx.enter_context(tc.tile_pool(name="dram", bufs=1, space="DRAM"))

        # persistent SBUF tensors
        lnT = singles.tile([P, ECH, S], dt.bfloat16)          # ln^T, e-chunked
        qkT = singles.tile([P, 4, S], dt.bfloat16)            # fc 0,1: Q^T; 2,3: K^T
        # token-major V (+ ones col at 64) per t-chunk / head; 68 pad so the
        # DoubleRow k-pair stride (4*68) is a multiple of 16 bytes
        Vp = singles.tile([P, TC, HPC, 68], EDT)
        # exp staging: (pr, kcp, head, parity, q); parity = kc&1 so a kcp
        # slice is the [Ki, Ko=2, N] moving operand of the DoubleRow ctx mm
        eP = singles.tile([P, 2, NKP, 2, 2, QTS], EDT)
        ones_dr = singles.tile([P, 2, 1], EDT)                # denominator lhsT
        w_sb = singles.tile([P, ECH, 3 * FQK], dt.bfloat16)
        wo_sb = singles.tile([P, 2, E], dt.bfloat16)
        eps_sb = singles.tile([P, 1], dt.float32)
        ctxn = singles.tile([P, 2, S], dt.bfloat16)           # normalized ctx^T
        ln_dram = dram.tile([S, E], dt.bfloat16)
        rc_dram = dram.tile([NQT * 4, QTS], dt.float32)

        shf_sb = singles.tile([P, 1], dt.float32)
        nc.vector.memset(shf_sb[:], EXP_SHIFT)
        nc.vector.memset(eps_sb[:], EPS)
        nc.sync.dma_start(w_sb[:], wqkv_d.rearrange("(c p) f -> p c f", p=P))
        nc.sync.dma_start(wo_sb[:], wo_d.rearrange("(c p) e -> p c e", p=P))
        nc.vector.memset(Vp[:], 0.0)
        nc.vector.memset(Vp[:, :, :, 64:65], 1.0)
        nc.vector.memset(ones_dr[:], 1.0)

        # ---- front: LN -> transpose -> QKV ------------------------------
        # Emission order is engineered around the per-engine FIFO queues:
        #   - ALL LN chunks first, so the DVE queue is a clean LN pipeline
        #     (an evacuation emitted mid-LN would stall later LN chunks
        #     behind the whole transpose->QKV chain)
        #   - transposes per superchunk on the sync queue (dispatch-bound,
        #     ~9us/MB: they get the queue to themselves)
        #   - ln bounce writes on the SWDGE queue, x loads on the scalar
        #     queue, each free-running
        #   - QKV per superchunk afterwards, with round-0 scores+exp
        #     hoisted between superchunk groups so ACT starts ~25us in
        # scores psum: [h_even | h_odd] per kc, 2 banks each, double-buffered.
        # Opened before psF (4 + 4 = 8 banks during the front; psF's banks
        # are recycled into the ctx/outproj pools afterwards).
        psS = ctx.enter_context(tc.tile_pool(name="psS", bufs=2, space="PSUM"))

        def scores_exp(pr, qt, kc):
            q0 = qt * QTS
            k0 = kc * P
            kcp, par = divmod(kc, 2)
            sq = psS.tile([P, 2 * QTS], dt.float32, tag="sq", name="sq")
            nc.tensor.matmul(
                sq[:, 0:QTS],
                qkT[0:64, 2 + pr, k0 : k0 + P],
                qkT[0:64, pr, q0 : q0 + QTS],
                start=True, stop=True, tile_position=(0, 0),
            )
            nc.tensor.matmul(
                sq[:, QTS : 2 * QTS],
                qkT[64:128, 2 + pr, k0 : k0 + P],
                qkT[64:128, pr, q0 : q0 + QTS],
                start=True, stop=True, tile_position=(64, 0),
            )
            # exp(s - C): softmax is shift-invariant (the ones-column
            # denominator uses the same shifted values).  The shift keeps
            # the heavy score tail under fp8e4m3's 448 max; the underflow
            # of tiny weights costs <0.2% of the denominator mass.
            nc.scalar.activation(
                eP[:, pr, kcp, :, par, :],
                sq[:].rearrange("p (h q) -> p h q", h=2),
                Act.Exp,
                bias=shf_sb[:],
            )

        with tc.tile_pool(name="psF", bufs=2, space="PSUM") as psF:
            # HAM warmup while LN(sc0) runs on DVE/DMA
            wps = psF.tile([P, 512], dt.float32, tag="qkv", name="wps")
            for _ in range(36):
                nc.tensor.matmul(
                    wps[:], w_sb[:, 0, 0:P], w_sb[:, 0, 0:512],
                    start=True, stop=True,
                )

            for sc in range(NSC):
                t0, t1 = sc * (TC // NSC), (sc + 1) * (TC // NSC)
                ln_writes = []
                for t in range(t0, t1):
                    xb = xp.tile([P, E], dt.float32, tag="xb")
                    nc.scalar.dma_start(xb[:], x_d[t * P : (t + 1) * P, :])
                    st = small.tile([P, 2, 6], dt.float32, tag="st")
                    nc.vector.bn_stats(st[:, 0, :], xb[:, 0:512])
                    nc.vector.bn_stats(st[:, 1, :], xb[:, 512:1024])
                    mv = small.tile([P, 2], dt.float32, tag="mv")
                    nc.vector.bn_aggr(mv[:], st[:])
                    sd = small.tile([P, 1], dt.float32, tag="sd")
                    nc.scalar.activation(sd[:], mv[:, 1:2], Act.Sqrt, bias=eps_sb[:])
                    rs = small.tile([P, 1], dt.float32, tag="rs")
                    nc.vector.reciprocal(rs[:], sd[:])
                    lnb = lnp.tile([P, E], dt.bfloat16, tag="lnb")
                    nc.vector.tensor_scalar(
                        lnb[:], xb[:], mv[:, 0:1], rs[:], Alu.subtract, Alu.mult
                    )
                    ln_writes.append(
                        nc.gpsimd.dma_start(ln_dram[t * P : (t + 1) * P, :], lnb[:])
                    )
                # transpose this superchunk (DRAM pool tiles are not
                # dependency-tracked; add the RAW edges explicitly)
                for c in range(ECH):
                    tp = nc.sync.dma_start_transpose(
                        lnT[:, c, sc * SCT : (sc + 1) * SCT],
                        ln_dram[sc * SCT : (sc + 1) * SCT, c * P : (c + 1) * P],
                    )
                    for wi in ln_writes:
                        add_dep_helper(tp.ins, wi.ins, True, "lnT RAW via ln_dram")

            def qkv_sc(sc):
                t0, t1 = sc * (TC // NSC), (sc + 1) * (TC // NSC)
                # K then Q, feature-major, weights-stationary
                for fc in (2, 3, 0, 1):
                    ps = psF.tile([P, 512], dt.float32, tag="qkv")
                    for ec in range(ECH):
                        nc.tensor.matmul(
                            ps[:],
                            w_sb[:, ec, fc * P : (fc + 1) * P],
                            lnT[:, ec, sc * SCT : (sc + 1) * SCT],
                            start=(ec == 0),
                            stop=(ec == ECH - 1),
                        )
                    nc.vector.tensor_copy(qkT[:, fc, sc * SCT : (sc + 1) * SCT], ps[:])
                # V token-major: lnT chunk stationary, V weight cols moving
                for t in range(t0, t1):
                    vps = psF.tile([P, FQK], dt.float32, tag="vtm")
                    for ec in range(ECH):
                        nc.tensor.matmul(
                            vps[:],
                            lnT[:, ec, t * P : (t + 1) * P],
                            w_sb[:, ec, 2 * FQK : 3 * FQK],
                            start=(ec == 0),
                            stop=(ec == ECH - 1),
                        )
                    nc.vector.tensor_copy(
                        Vp[:, t, :, 0:64],
                        vps[:].rearrange("p (h d) -> p h d", d=64),
                    )

            # QKV per superchunk, round-0 scores+exp hoisted in between:
            # after sc's K/Q exist, the kc chunks covered by sc are feasible
            for sc in range(NSC):
                qkv_sc(sc)
                if sc < NSC - 1:
                    for kc in range(4 * sc, 4 * sc + 4):
                        scores_exp(0, 0, kc)
                        scores_exp(1, 0, kc)

        # ---- attention: per q-tile, both head-pairs' exp interleaved ------
        ctx2 = ExitStack()
        with ctx2:
            psC = ctx2.enter_context(tc.tile_pool(name="psC", bufs=1, space="PSUM"))
            psO = ctx2.enter_context(tc.tile_pool(name="psO", bufs=1, space="PSUM"))

            def ctx_block(pr, hh_base):
                # 8 DoubleRow matmuls per head: 2 key-chunks per instruction;
                # row 64 of the output accumulates the softmax denominator
                cps = []
                for h in range(2):
                    cp = psC.tile([65, QTS], dt.float32, tag=f"cps{h}",
                                  name=f"cps{h}")
                    cps.append(cp)
                    for kcp in range(NKP):
                        if CTX_DR:
                            nc.tensor.matmul(
                                cp[:],
                                Vp[:, 2 * kcp : 2 * kcp + 2, hh_base + h, 0:65],
                                eP[:, pr, kcp, h, :, :],
                                start=(kcp == 0), stop=(kcp == NKP - 1),
                                perf_mode=DR,
                            )
                        else:
                            for par in range(2):
                                nc.tensor.matmul(
                                    cp[:],
                                    Vp[:, 2 * kcp + par, hh_base + h, 0:65],
                                    eP[:, pr, kcp, h, par, :],
                                    start=(kcp == 0 and par == 0),
                                    stop=(kcp == NKP - 1 and par == 1),
                                )
                return cps

            def normalize(pr, qt, cps):
                # ctx^T[d,q] / denom[q]; denom is ctx row 64.  reciprocal of
                # a [1,512] row is ~3us on one DVE lane, so reshape to
                # [128,4] via a small DMA, recip, then broadcast across 64
                # partitions with a step-0 DMA read from DRAM.
                q0 = qt * QTS
                ctxu = []
                for h in range(2):
                    cu = evac.tile([65, QTS], dt.float32, tag=f"ctxu{h}",
                                   name=f"ctxu{h}")
                    nc.vector.tensor_copy(cu[:], cps[h][:])
                    ctxu.append(cu)
                for h in range(2):
                    dnp = small.tile([P, QTS // P], dt.float32, tag="dnp")
                    nc.sync.dma_start(dnp[:], ctxu[h][64:65, :])
                    rcp = small.tile([P, QTS // P], dt.float32, tag="rcp")
                    nc.vector.reciprocal(rcp[:], dnp[:])
                    slot = (qt * 2 + pr) * 2 + h
                    rc_row = rc_dram[slot : slot + 1, :]
                    wr = nc.sync.dma_start(rc_row, rcp[:])
                    bcs = evac.tile([64, QTS], dt.float32, tag="bcs")
                    rc_bcast = bass.AP(
                        tensor=rc_row.tensor,
                        offset=rc_row.offset,
                        ap=[[0, 64]] + list(rc_row.ap[1:]),
                    )
                    rd = nc.sync.dma_start(bcs[:], rc_bcast)
                    add_dep_helper(rd.ins, wr.ins, True, "recip RAW via dram")
                    if h == 0:
                        nc.vector.tensor_tensor(
                            ctxn[0:64, pr, q0 : q0 + QTS],
                            ctxu[0][0:64, :], bcs[:], Alu.mult,
                        )
                    else:
                        tmpn = evac.tile([64, QTS], dt.bfloat16, tag="tmpn")
                        nc.vector.tensor_tensor(
                            tmpn[:], ctxu[1][0:64, :], bcs[:], Alu.mult
                        )
                        # partition shift 0-63 -> 64-127 via SBUF-SBUF DMA
                        nc.sync.dma_start(ctxn[64:128, pr, q0 : q0 + QTS], tmpn[:])

            def outproj(qt):
                for t in range(qt * (QTS // P), (qt + 1) * (QTS // P)):
                    po = psO.tile([P, E], dt.float32, tag="po")
                    for et in range(2):
                        for pr in range(2):
                            nc.tensor.matmul(
                                po[:, et * 512 : (et + 1) * 512],
                                ctxn[:, pr, t * P : (t + 1) * P],
                                wo_sb[:, pr, et * 512 : (et + 1) * 512],
                                start=(pr == 0), stop=(pr == 1),
                            )
                    ob = evac.tile([P, E], dt.float32, tag="ob", bufs=3)
                    nc.vector.tensor_copy(ob[:], po[:])
                    nc.scalar.dma_start(out_d[t * P : (t + 1) * P, :], ob[:])

            # outproj(qt) is EMITTED a few kc into round qt+1: the PE queue
            # is a static FIFO, and outproj's ctxn dependency sits behind a
            # multi-DMA normalize chain -- placing it before the next
            # round's scores would stall the scores and starve ACT
            pend = None
            for qt in range(NQT):
                # round 0's kc 0-11 were hoisted into the front
                for kc in range(12 if qt == 0 else 0, TC):
                    scores_exp(0, qt, kc)
                    scores_exp(1, qt, kc)
                    if kc == 5 and pend is not None:
                        outproj(pend)
                        pend = None
                for pr in range(2):
                    cps = ctx_block(pr, pr * 2)
                    normalize(pr, qt, cps)
                pend = qt
            outproj(pend)

    nc.compile()
    return nc


def make_in_maps(x, ln_scale, w_qkv, w_out):
    w = (np.asarray(w_qkv, np.float32) * np.asarray(ln_scale, np.float32)[:, None])
    wo = np.asarray(w_out, np.float32)
    in_maps = []
    for c in range(NCORES):
        b, g = divmod(c, 4)
        h0 = g * HPC
        wq = w[:, h0 * D : (h0 + HPC) * D]
        wk = w[:, H * D + h0 * D : H * D + (h0 + HPC) * D]
        wv = w[:, 2 * H * D + h0 * D : 2 * H * D + (h0 + HPC) * D]
        in_maps.append(
            {
                "x": np.ascontiguousarray(np.asarray(x, np.float32)[:, b, :]),
                "wqkv": np.ascontiguousarray(
                    np.concatenate([wq, wk, wv], axis=1)
                ).astype(BF16),
                "wo": np.ascontiguousarray(
                    wo[h0 * D : (h0 + HPC) * D, :]
                ).astype(BF16),
            }
        )
    return in_maps


def get_nc():
    if "nc" not in _CACHE:
        _CACHE["nc"] = _build_nc()
    return _CACHE["nc"]


def assemble(results):
    out = np.empty((S, B, E), np.float32)
    for b in range(B):
        acc = results[4 * b]["out"].astype(np.float32).copy()
        for g in range(1, 4):
            acc += results[4 * b + g]["out"]
        out[:, b, :] = acc
    return out


def kernel(x, ln_scale, w_qkv, w_out):
    from concourse.bass_utils import run_bass_kernel_spmd

    nc = get_nc()
    in_maps = make_in_maps(x, ln_scale, w_qkv, w_out)
    res = run_bass_kernel_spmd(nc, in_maps, core_ids=list(range(NCORES)))
    return assemble(res.results)


# revision 19
# speedup vs baseline: 1.0467x; 1.0018x over previous
# Trainium2 Bass kernel for nn_MultiHeadAttention_85933705658435
#
# Reference: LayerNorm(scale-only) -> QKV projection -> full softmax attention
#            -> output projection.  x:[S,B,E] f32, S=2048 B=2 E=1024, H=16 D=64.
#
# Sharding (8 cores): batch (2-way) x head-group (4-way, 4 heads/core).
#   - W_qkv column-sharded (the core's 4 heads), W_out row-sharded; the 4
#     partial output projections per batch are summed on the host.
#
# v2 structure (vs the 319us baseline):
#   - LN -> DMA-transpose -> QKV runs as a per-512-token-superchunk pipeline,
#     so the PE starts QKV ~12us in instead of idling 67us behind a fully
#     serialized LN phase.
#   - V is computed token-major directly (lnT chunk as the stationary
#     operand), killing the V DRAM bounce + 16 transposes of the baseline.
#   - Attention is emitted per 512-query tile with the two head-pairs'
#     scores+exp interleaved, so ACT (the exp engine, the true bottleneck at
#     ~1.15us per [128,1024] exp) starts as soon as superchunk 0's K/Q exist
#     and never waits for the full QKV phase.
#   - exp writes fp8e4 pairs; ctx uses fp8 DoubleRow matmuls (2 key-chunks
#     contracted per instruction) with the appended-ones column still giving
#     the softmax denominators for free.
#   - out-projection of q-tile qt overlaps the attention of qt+1 (its own
#     2 PSUM banks; scores 4 + ctx 2 + outproj 2 = 8 banks exactly).

import numpy as np
import ml_dtypes

S, B, E = 2048, 2, 1024
H, D = 16, 64
HPC = 4              # heads per core
NCORES = 8
EPS = 1e-6
FQK = HPC * D        # 256 (per-core Q width = K width = V width)
P = 128
TC = S // P          # 16 token chunks
ECH = E // P         # 8 e-chunks
NSC = 4              # superchunks in the front pipeline
SCT = S // NSC       # 512 tokens per superchunk
QTS = 512            # q-tile size in attention
NQT = S // QTS       # 4
NKP = TC // 2        # 8 key-chunk pairs

CTX_FP8 = False      # fp8e4 exp + V'
CTX_DR = False       # DoubleRow ctx matmuls (requires CTX_FP8)
EXP_SHIFT = -1.6 if CTX_FP8 else 0.0

BF16 = ml_dtypes.bfloat16

_CACHE = {}


def _build_nc():
    from contextlib import ExitStack

    import concourse.bass as bass
    import concourse.tile as tile
    from concourse import bacc, mybir
    from concourse.tile import add_dep_helper

    dt = mybir.dt
    Alu = mybir.AluOpType
    Act = mybir.ActivationFunctionType
    EDT = dt.float8e4 if CTX_FP8 else dt.bfloat16
    DR = mybir.MatmulPerfMode.DoubleRow

    nc = bacc.Bacc(trn_type="TRN2")
    x_d = nc.dram_tensor("x", (S, E), dt.float32, kind="ExternalInput").ap()
    # wqkv: [E, 3*FQK] = Q | K | V column blocks for this core's 4 heads
    wqkv_d = nc.dram_tensor(
        "wqkv", (E, 3 * FQK), dt.bfloat16, kind="ExternalInput"
    ).ap()
    wo_d = nc.dram_tensor("wo", (FQK, E), dt.bfloat16, kind="ExternalInput").ap()
    out_d = nc.dram_tensor("out", (S, E), dt.float32, kind="ExternalOutput").ap()

    with tile.TileContext(nc) as tc, ExitStack() as ctx:
        singles = ctx.enter_context(tc.tile_pool(name="singles", bufs=1))
        xp = ctx.enter_context(tc.tile_pool(name="xp", bufs=3))
        lnp = ctx.enter_context(tc.tile_pool(name="lnp", bufs=3))
        small = ctx.enter_context(tc.tile_pool(name="small", bufs=4))
        evac = ctx.enter_context(tc.tile_pool(name="evac", bufs=2))
        dram = ctx.enter_context(tc.tile_pool(name="dram", bufs=1, space="DRAM"))

        # persistent SBUF tensors
        lnT = singles.tile([P, ECH, S], dt.bfloat16)          # ln^T, e-chunked
        qkT = singles.tile([P, 4, S], dt.bfloat16)            # fc 0,1: Q^T; 2,3: K^T
        # token-major V (+ ones col at 64) per t-chunk / head; 68 pad so the
        # DoubleRow k-pair stride (4*68) is a multiple of 16 bytes
        Vp = singles.tile([P, TC, HPC, 68], EDT)
        # exp staging: (pr, kcp, head, parity, q); parity = kc&1 so a kcp
        # slice is the [Ki, Ko=2, N] moving operand of the DoubleRow ctx mm
        eP = singles.tile([P, 2, NKP, 2, 2, QTS], EDT)
        ones_dr = singles.tile([P, 2, 1], EDT)                # denominator lhsT
        w_sb = singles.tile([P, ECH, 3 * FQK], dt.bfloat16)
        wo_sb = singles.tile([P, 2, E], dt.bfloat16)
        eps_sb = singles.tile([P, 1], dt.float32)
        ctxn = singles.tile([P, 2, S], dt.bfloat16)           # normalized ctx^T
        ln_dram = dram.tile([S, E], dt.bfloat16)
        rc_dram = dram.tile([NQT * 4, QTS], dt.float32)

        shf_sb = singles.tile([P, 1], dt.float32)
        nc.vector.memset(shf_sb[:], EXP_SHIFT)
        nc.vector.memset(eps_sb[:], EPS)
        nc.sync.dma_start(w_sb[:], wqkv_d.rearrange("(c p) f -> p c f", p=P))
        nc.sync.dma_start(wo_sb[:], wo_d.rearrange("(c p) e -> p c e", p=P))
        nc.vector.memset(Vp[:], 0.0)
        nc.vector.memset(Vp[:, :, :, 64:65], 1.0)
        nc.vector.memset(ones_dr[:], 1.0)

        # ---- front: LN -> transpose -> QKV ------------------------------
        # Emission order is engineered around the per-engine FIFO queues:
        #   - ALL LN chunks first, so the DVE queue is a clean LN pipeline
        #     (an evacuation emitted mid-LN would stall later LN chunks
        #     behind the whole transpose->QKV chain)
        #   - transposes per superchunk on the sync queue (dispatch-bound,
        #     ~9us/MB: they get the queue to themselves)
        #   - ln bounce writes on the SWDGE queue, x loads on the scalar
        #     queue, each free-running
        #   - QKV per superchunk afterwards, with round-0 scores+exp
        #     hoisted between superchunk groups so ACT starts ~25us in
        # scores psum: [h_even | h_odd] per kc, 2 banks each, double-buffered.
        # Opened before psF (4 + 4 = 8 banks during the front; psF's banks
        # are recycled into the ctx/outproj pools afterwards).
        psS = ctx.enter_context(tc.tile_pool(name="psS", bufs=2, space="PSUM"))

        def scores_exp(pr, qt, kc):
            q0 = qt * QTS
            k0 = kc * P
            kcp, par = divmod(kc, 2)
            sq = psS.tile([P, 2 * QTS], dt.float32, tag="sq", name="sq")
            nc.tensor.matmul(
                sq[:, 0:QTS],
                qkT[0:64, 2 + pr, k0 : k0 + P],
                qkT[0:64, pr, q0 : q0 + QTS],
                start=True, stop=True, tile_position=(0, 0),
            )
            nc.tensor.matmul(
                sq[:, QTS : 2 * QTS],
                qkT[64:128, 2 + pr, k0 : k0 + P],
                qkT[64:128, pr, q0 : q0 + QTS],
                start=True, stop=True, tile_position=(64, 0),
            )
            # exp(s - C): softmax is shift-invariant (the ones-column
            # denominator uses the same shifted values).  The shift keeps
            # the heavy score tail under fp8e4m3's 448 max; the underflow
            # of tiny weights costs <0.2% of the denominator mass.
            nc.scalar.activation(
                eP[:, pr, kcp, :, par, :],
                sq[:].rearrange("p (h q) -> p h q", h=2),
                Act.Exp,
                bias=shf_sb[:],
            )

        with tc.tile_pool(name="psF", bufs=2, space="PSUM") as psF:
            # HAM warmup while LN(sc0) runs on DVE/DMA
            wps = psF.tile([P, 512], dt.float32, tag="qkv", name="wps")
            for _ in range(36):
                nc.tensor.matmul(
                    wps[:], w_sb[:, 0, 0:P], w_sb[:, 0, 0:512],
                    start=True, stop=True,
                )

            for sc in range(NSC):
                t0, t1 = sc * (TC // NSC), (sc + 1) * (TC // NSC)
                ln_writes = []
                for t in range(t0, t1):
                    xb = xp.tile([P, E], dt.float32, tag="xb")
                    nc.scalar.dma_start(xb[:], x_d[t * P : (t + 1) * P, :])
                    st = small.tile([P, 2, 6], dt.float32, tag="st")
                    nc.vector.bn_stats(st[:, 0, :], xb[:, 0:512])
                    nc.vector.bn_stats(st[:, 1, :], xb[:, 512:1024])
                    mv = small.tile([P, 2], dt.float32, tag="mv")
                    nc.vector.bn_aggr(mv[:], st[:])
                    sd = small.tile([P, 1], dt.float32, tag="sd")
                    nc.scalar.activation(sd[:], mv[:, 1:2], Act.Sqrt, bias=eps_sb[:])
                    rs = small.tile([P, 1], dt.float32, tag="rs")
                    nc.vector.reciprocal(rs[:], sd[:])
                    lnb = lnp.tile([P, E], dt.bfloat16, tag="lnb")
                    nc.vector.tensor_scalar(
                        lnb[:], xb[:], mv[:, 0:1], rs[:], Alu.subtract, Alu.mult
                    )
                    # ln bounce on sync (shared with the transposes, which
                    # consume them in FIFO order anyway); x loads stay on the
                    # scalar queue so they free-run ahead
                    ln_writes.append(
                        nc.sync.dma_start(ln_dram[t * P : (t + 1) * P, :], lnb[:])
                    )
                # transpose this superchunk (DRAM pool tiles are not
                # dependency-tracked; add the RAW edges explicitly)
                for c in range(ECH):
                    tp = nc.sync.dma_start_transpose(
                        lnT[:, c, sc * SCT : (sc + 1) * SCT],
                        ln_dram[sc * SCT : (sc + 1) * SCT, c * P : (c + 1) * P],
                    )
                    for wi in ln_writes:
                        add_dep_helper(tp.ins, wi.ins, True, "lnT RAW via ln_dram")

            def qkv_sc(sc):
                t0, t1 = sc * (TC // NSC), (sc + 1) * (TC // NSC)
                # K then Q, feature-major, weights-stationary
                for fc in (2, 3, 0, 1):
                    ps = psF.tile([P, 512], dt.float32, tag="qkv")
                    for ec in range(ECH):
                        nc.tensor.matmul(
                            ps[:],
                            w_sb[:, ec, fc * P : (fc + 1) * P],
                            lnT[:, ec, sc * SCT : (sc + 1) * SCT],
                            start=(ec == 0),
                            stop=(ec == ECH - 1),
                        )
                    nc.vector.tensor_copy(qkT[:, fc, sc * SCT : (sc + 1) * SCT], ps[:])
                # V token-major: lnT chunk stationary, V weight cols moving
                for t in range(t0, t1):
                    vps = psF.tile([P, FQK], dt.float32, tag="vtm")
                    for ec in range(ECH):
                        nc.tensor.matmul(
                            vps[:],
                            lnT[:, ec, t * P : (t + 1) * P],
                            w_sb[:, ec, 2 * FQK : 3 * FQK],
                            start=(ec == 0),
                            stop=(ec == ECH - 1),
                        )
                    nc.vector.tensor_copy(
                        Vp[:, t, :, 0:64],
                        vps[:].rearrange("p (h d) -> p h d", d=64),
                    )

            # QKV per superchunk, round-0 scores+exp hoisted in between:
            # after sc's K/Q exist, the kc chunks covered by sc are feasible
            for sc in range(NSC):
                qkv_sc(sc)
                if sc < NSC - 1:
                    for kc in range(4 * sc, 4 * sc + 4):
                        scores_exp(0, 0, kc)
                        scores_exp(1, 0, kc)

        # ---- attention: per q-tile, both head-pairs' exp interleaved ------
        ctx2 = ExitStack()
        with ctx2:
            psC = ctx2.enter_context(tc.tile_pool(name="psC", bufs=1, space="PSUM"))
            psO = ctx2.enter_context(tc.tile_pool(name="psO", bufs=1, space="PSUM"))

            def ctx_block(pr, hh_base):
                # 8 DoubleRow matmuls per head: 2 key-chunks per instruction;
                # row 64 of the output accumulates the softmax denominator
                cps = []
                for h in range(2):
                    cp = psC.tile([65, QTS], dt.float32, tag=f"cps{h}",
                                  name=f"cps{h}")
                    cps.append(cp)
                    for kcp in range(NKP):
                        if CTX_DR:
                            nc.tensor.matmul(
                                cp[:],
                                Vp[:, 2 * kcp : 2 * kcp + 2, hh_base + h, 0:65],
                                eP[:, pr, kcp, h, :, :],
                                start=(kcp == 0), stop=(kcp == NKP - 1),
                                perf_mode=DR,
                            )
                        else:
                            for par in range(2):
                                nc.tensor.matmul(
                                    cp[:],
                                    Vp[:, 2 * kcp + par, hh_base + h, 0:65],
                                    eP[:, pr, kcp, h, par, :],
                                    start=(kcp == 0 and par == 0),
                                    stop=(kcp == NKP - 1 and par == 1),
                                )
                return cps

            def normalize(pr, qt, cps):
                # ctx^T[d,q] / denom[q]; denom is ctx row 64.  reciprocal of
                # a [1,512] row is ~3us on one DVE lane, so reshape to
                # [128,4] via a small DMA, recip, then broadcast across 64
                # partitions with a step-0 DMA read from DRAM.
                q0 = qt * QTS
                ctxu = []
                for h in range(2):
                    cu = evac.tile([65, QTS], dt.float32, tag=f"ctxu{h}",
                                   name=f"ctxu{h}")
                    nc.vector.tensor_copy(cu[:], cps[h][:])
                    ctxu.append(cu)
                for h in range(2):
                    dnp = small.tile([P, QTS // P], dt.float32, tag="dnp")
                    nc.sync.dma_start(dnp[:], ctxu[h][64:65, :])
                    rcp = small.tile([P, QTS // P], dt.float32, tag="rcp")
                    nc.vector.reciprocal(rcp[:], dnp[:])
                    slot = (qt * 2 + pr) * 2 + h
                    rc_row = rc_dram[slot : slot + 1, :]
                    wr = nc.sync.dma_start(rc_row, rcp[:])
                    bcs = evac.tile([64, QTS], dt.float32, tag="bcs")
                    rc_bcast = bass.AP(
                        tensor=rc_row.tensor,
                        offset=rc_row.offset,
                        ap=[[0, 64]] + list(rc_row.ap[1:]),
                    )
                    rd = nc.sync.dma_start(bcs[:], rc_bcast)
                    add_dep_helper(rd.ins, wr.ins, True, "recip RAW via dram")
                    if h == 0:
                        nc.vector.tensor_tensor(
                            ctxn[0:64, pr, q0 : q0 + QTS],
                            ctxu[0][0:64, :], bcs[:], Alu.mult,
                        )
                    else:
                        tmpn = evac.tile([64, QTS], dt.bfloat16, tag="tmpn")
                        nc.vector.tensor_tensor(
                            tmpn[:], ctxu[1][0:64, :], bcs[:], Alu.mult
                        )
                        # partition shift 0-63 -> 64-127 via SBUF-SBUF DMA
                        nc.sync.dma_start(ctxn[64:128, pr, q0 : q0 + QTS], tmpn[:])

            def outproj(qt):
                for t in range(qt * (QTS // P), (qt + 1) * (QTS // P)):
                    po = psO.tile([P, E], dt.float32, tag="po")
                    for et in range(2):
                        for pr in range(2):
                            nc.tensor.matmul(
                                po[:, et * 512 : (et + 1) * 512],
                                ctxn[:, pr, t * P : (t + 1) * P],
                                wo_sb[:, pr, et * 512 : (et + 1) * 512],
                                start=(pr == 0), stop=(pr == 1),
                            )
                    ob = evac.tile([P, E], dt.float32, tag="ob", bufs=3)
                    nc.vector.tensor_copy(ob[:], po[:])
                    # output writes go to the SWDGE queue: they are gated on
                    # the normalize chain, and on the scalar queue they would
                    # stall the exp stream behind them (FIFO)
                    nc.gpsimd.dma_start(out_d[t * P : (t + 1) * P, :], ob[:])

            # outproj(qt) is EMITTED a few kc into round qt+1: the PE queue
            # is a static FIFO, and outproj's ctxn dependency sits behind a
            # multi-DMA normalize chain -- placing it before the next
            # round's scores would stall the scores and starve ACT
            pend = None
            for qt in range(NQT):
                # round 0's kc 0-11 were hoisted into the front
                for kc in range(12 if qt == 0 else 0, TC):
                    scores_exp(0, qt, kc)
                    scores_exp(1, qt, kc)
                    if kc == 5 and pend is not None:
                        outproj(pend)
                        pend = None
                for pr in range(2):
                    cps = ctx_block(pr, pr * 2)
                    normalize(pr, qt, cps)
                pend = qt
            outproj(pend)

    nc.compile()
    return nc


def make_in_maps(x, ln_scale, w_qkv, w_out):
    w = (np.asarray(w_qkv, np.float32) * np.asarray(ln_scale, np.float32)[:, None])
    wo = np.asarray(w_out, np.float32)
    in_maps = []
    for c in range(NCORES):
        b, g = divmod(c, 4)
        h0 = g * HPC
        wq = w[:, h0 * D : (h0 + HPC) * D]
        wk = w[:, H * D + h0 * D : H * D + (h0 + HPC) * D]
        wv = w[:, 2 * H * D + h0 * D : 2 * H * D + (h0 + HPC) * D]
        in_maps.append(
            {
                "x": np.ascontiguousarray(np.asarray(x, np.float32)[:, b, :]),
                "wqkv": np.ascontiguousarray(
                    np.concatenate([wq, wk, wv], axis=1)
                ).astype(BF16),
                "wo": np.ascontiguousarray(
                    wo[h0 * D : (h0 + HPC) * D, :]
                ).astype(BF16),
            }
        )
    return in_maps


def get_nc():
    if "nc" not in _CACHE:
        _CACHE["nc"] = _build_nc()
    return _CACHE["nc"]


def assemble(results):
    out = np.empty((S, B, E), np.float32)
    for b in range(B):
        acc = results[4 * b]["out"].astype(np.float32).copy()
        for g in range(1, 4):
            acc += results[4 * b + g]["out"]
        out[:, b, :] = acc
    return out


def kernel(x, ln_scale, w_qkv, w_out):
    from concourse.bass_utils import run_bass_kernel_spmd

    nc = get_nc()
    in_maps = make_in_maps(x, ln_scale, w_qkv, w_out)
    res = run_bass_kernel_spmd(nc, in_maps, core_ids=list(range(NCORES)))
    return assemble(res.results)
